# revision 42
# baseline (speedup 1.0000x reference)
"""Trainium2 Bass kernel for DistanceBasedAttention (L1-distance attention).

Contract: kernel(**inputs) takes FULL unsharded inputs (as produced by
setup_inputs()) and returns the FULL output [B, S, HID] float32.

Sharding: the 16 (batch, head) blocks are split 2-per-core across 8 cores
(core = b*4 + head_pair). Each core computes its two heads' attention output
and the partial out-projection (Wo rows of its heads); the host sums the four
per-batch partials and adds the effective bias (bv @ Wo + bo).

Math: with SC = LAMBDA/sqrt(HD),
    D[j,i] = sum_d |q_id - k_jd| = 2*sum_d relu(q_id - k_jd) - Qsum[i] + Ksum[j]
-Qsum[i] is constant along the softmax axis and cancels; Ksum[j] is folded into
the exp's per-partition bias column (host pre-scales the Ksum weights by -SC).

relu A-tiles [128 = 64d x 2keys, 512 queries] are reduced over d by
TensorEngine matmuls with 0/2 selector weights. Tiles are routed between
two pipelines, balancing accuracy (fp8 adds ~quantization noise) and speed:
  - fp16: produced on DVE (fast 4x mode), consumed by [32-row] fp16 matmuls
    at tile_position (0,32g), one stream of 512 cols per 2 keys.
  - fp8(e4m3): produced on Pool(gpsimd)/ACT/DVE, pairs consumed by full-width
    DoubleRow matmuls (dst partition must be 0 on HW) -> 4 keys per 256-cycle
    stream (4x the fp16 key rate).
The softmax denominator is folded into the AV matmul via a ones-column block
in the V tile ([av|ones|av] layout -> cs rows come out of the same matmuls).
"""

import numpy as np

import concourse.bass as bass
import concourse.mybir as mybir
import concourse.tile as tile
from concourse.bass_utils import run_bass_kernel_spmd

F32 = mybir.dt.float32
F16 = mybir.dt.float16
F8 = mybir.dt.float8e4
Alu = mybir.AluOpType
Act = mybir.ActivationFunctionType
DR = mybir.MatmulPerfMode.DoubleRow

B, S, HID = 2, 512, 512
NH, HD = 8, 64
LAMBDA = 1.0
SCALE = float(LAMBDA / np.sqrt(HD))
N_CORES = 8

import os as _os

# fraction of tile-slots routed through the fp8 DoubleRow path
F8_NUM = int(_os.environ.get("DBA_F8_NUM", "8"))
F8_DEN = int(_os.environ.get("DBA_F8_DEN", "16"))
# greedy engine-balance costs (ns) for fp8 sub-tile producers
COST_POOL = float(_os.environ.get("DBA_COST_POOL", "540"))
COST_ACT = float(_os.environ.get("DBA_COST_ACT", "640"))
COST_DVE8 = float(_os.environ.get("DBA_COST_DVE8", "600"))
COST_DVE16 = float(_os.environ.get("DBA_COST_DVE16", "200"))
# initial engine load offsets (ns): fixed work emitted outside the banks
LOAD0_ACT = float(_os.environ.get("DBA_LOAD0_ACT", "5500"))
LOAD0_POOL = float(_os.environ.get("DBA_LOAD0_POOL", "300"))
LOAD0_DVE = float(_os.environ.get("DBA_LOAD0_DVE", "6100"))
WORK_BUFS = int(_os.environ.get("DBA_WORK_BUFS", "16"))
PBANK_BUFS = int(_os.environ.get("DBA_PBANK_BUFS", "4"))
# debug kill-switches
SEL8_SP = _os.environ.get("DBA_SEL8_SP", "") == "1"
KTP_ACT = _os.environ.get("DBA_KTP_ACT", "") == "1"
# interleaving an open PSUM accumulation group (av) with other start/stop
# groups NaNs on real HW (sim doesn't model it) -- keep av contiguous
AV_LATE = _os.environ.get("DBA_AV_LATE", "1") == "1"


def _split_excess_waits(nc, max_waits=1):
    """walrus in this container accepts a single sync-wait per instruction;
    move excess waits onto same-engine NoOps inserted just before."""
    f = nc.m.functions[0]
    for bb in f.blocks:
        new_list = []
        changed = False
        for ins in bb.instructions:
            si = ins.sync_info
            if si is not None and si.on_wait is not None and len(si.on_wait) > max_waits:
                waits = list(si.on_wait)
                k = 0
                while len(waits) - k > max_waits:
                    chunk = waits[k : k + max_waits]
                    k += max_waits
                    nop = mybir.InstNoOp(name=f"{ins.name}-ws-{k}", ins=[], outs=[])
                    nop.engine = ins.engine
                    nop.sync_info = mybir.SyncInfo(on_wait=chunk, on_update=[])
                    new_list.append(nop)
                si.on_wait = waits[k:]
                changed = True
            new_list.append(ins)
        if changed:
            bb.instructions = new_list


def _build_program(repeat=0):
    nc = bass.Bass()
    hidt_d = nc.dram_tensor("hidt", [HID, S], F16, kind="ExternalInput")
    wq2_d = nc.dram_tensor("wq2", [HID, 256], F16, kind="ExternalInput")
    wk2n_d = nc.dram_tensor("wk2n", [HID, 128], F16, kind="ExternalInput")
    wv2_d = nc.dram_tensor("wv2", [HID, 128], F16, kind="ExternalInput")
    wo2_d = nc.dram_tensor("wo2", [128, HID], F16, kind="ExternalInput")
    wks2_d = nc.dram_tensor("wks2", [HID, 2], F16, kind="ExternalInput")
    bqcol_d = nc.dram_tensor("bqcol", [128, 2], F32, kind="ExternalInput")
    bkncol_d = nc.dram_tensor("bkncol", [128, 2], F32, kind="ExternalInput")
    bks2_d = nc.dram_tensor("bks2", [2, 1], F32, kind="ExternalInput")
    sel16_d = nc.dram_tensor("sel16", [128, 16, 32], F16, kind="ExternalInput")
    sel8_d = nc.dram_tensor("sel8", [128, 32, 2, 128], F8, kind="ExternalInput")
    eye2_d = nc.dram_tensor("eye2", [2, 2], F32, kind="ExternalInput")
    outp_d = nc.dram_tensor("outp", [S, HID], F32, kind="ExternalOutput")

    with tile.TileContext(nc) as tc:
        with (
            tc.tile_pool(name="consts", bufs=1) as consts,
            tc.tile_pool(name="work", bufs=WORK_BUFS) as work,
            tc.tile_pool(name="pbank", bufs=PBANK_BUFS, space="PSUM") as pbank,
            tc.tile_pool(name="paux", bufs=2, space="PSUM") as paux,
        ):
            if repeat:
                loop_cm = tc.For_i(
                    0, repeat, 1,
                    hint_engines=(
                        mybir.EngineType.DVE,
                        mybir.EngineType.Activation,
                        mybir.EngineType.PE,
                        mybir.EngineType.Pool,
                        mybir.EngineType.SP,
                    ),
                )
            else:
                import contextlib
                loop_cm = contextlib.nullcontext()
            with loop_cm:
                _emit_body(
                    nc, consts, work, pbank, paux,
                    hidt_d, wq2_d, wk2n_d, wv2_d, wo2_d, wks2_d,
                    bqcol_d, bkncol_d, bks2_d, sel16_d, sel8_d, eye2_d, outp_d,
                )

    _split_excess_waits(nc)
    return nc


def _emit_body(
    nc, consts, work, pbank, paux,
    hidt_d, wq2_d, wk2n_d, wv2_d, wo2_d, wks2_d,
    bqcol_d, bkncol_d, bks2_d, sel16_d, sel8_d, eye2_d, outp_d,
):
    # ---- DMAs: SP carries hidT + small tensors; ACT (idle early) carries the
    # wide weight/selector tensors so nothing gates the pipeline start ----
    wq2 = consts.tile([128, 4, 256], F16, name="wq2")
    nc.sync.dma_start(wq2[:], wq2_d.rearrange("(kt p) c -> p kt c", kt=4))
    hidT = consts.tile([128, 4, 512], F16, name="hidT")
    nc.sync.dma_start(hidT[:, 0, :], hidt_d[0:128, :])
    wk2n = consts.tile([128, 4, 128], F16, name="wk2n")
    nc.sync.dma_start(wk2n[:], wk2n_d.rearrange("(kt p) c -> p kt c", kt=4))
    sel8 = consts.tile([128, 32, 2, 128], F8, name="sel8")
    (nc.sync if SEL8_SP else nc.scalar).dma_start(sel8[:], sel8_d[:])
    for kt in range(1, 4):
        nc.sync.dma_start(hidT[:, kt, :], hidt_d[kt * 128 : (kt + 1) * 128, :])
    sel16 = consts.tile([128, 16, 32], F16, name="sel16")
    nc.sync.dma_start(sel16[:], sel16_d[:])
    bqcol = consts.tile([128, 2], F32, name="bqcol")
    nc.sync.dma_start(bqcol[:], bqcol_d[:])
    bkncol = consts.tile([128, 2], F32, name="bkncol")
    nc.sync.dma_start(bkncol[:], bkncol_d[:])
    wks2 = consts.tile([128, 4, 2], F16, name="wks2")
    nc.sync.dma_start(wks2[:], wks2_d.rearrange("(kt p) c -> p kt c", kt=4))
    bks2 = consts.tile([2, 1], F32, name="bks2")
    nc.sync.dma_start(bks2[:], bks2_d[:])
    eye2 = consts.tile([2, 2], F32, name="eye2")
    nc.sync.dma_start(eye2[:], eye2_d[:])
    wv2 = consts.tile([128, 4, 128], F16, name="wv2")
    nc.sync.dma_start(wv2[:], wv2_d.rearrange("(kt p) c -> p kt c", kt=4))
    wo2 = consts.tile([128, 512], F16, name="wo2")
    nc.sync.dma_start(wo2[:], wo2_d[:])
    # vext: [v_h0 (64) | ones (64) | v_h1 (64)] per key-tile
    vext = consts.tile([128, 4, 192], F16, name="vext")
    nc.gpsimd.memset(vext[:, :, 64:128], 1.0)

    def hidT_par(kt, par):
        return hidT[:, kt].rearrange("p (j two) -> p two j", two=2)[:, par, :]

    # ---- Q^T per head ----
    qt2 = []

    def emit_q(h):
        q_ps = pbank.tile([128, 512], F32, name="q_ps", tag="bank")
        for kt in range(4):
            nc.tensor.matmul(
                q_ps[:],
                wq2[:, kt, 128 * h : 128 * h + 128],
                hidT[:, kt, :],
                start=(kt == 0), stop=(kt == 3),
            )
        q_sb = consts.tile([128, 512], F16, name=f"qt2_{h}")
        if h == 0:
            nc.vector.tensor_scalar(
                q_sb[:], q_ps[:], bqcol[:, h : h + 1], None, Alu.add
            )
        else:
            nc.scalar.activation(
                q_sb[:], q_ps[:], Act.Identity, bias=bqcol[:, h : h + 1], scale=1.0
            )
        qt2.append(q_sb)

    # ---- -K^T both heads at once (lhsT = both heads' 64-col blocks) ----
    ktp = []

    def emit_k():
        kps = []
        for par in range(2):
            k_ps = pbank.tile([128, 256], F32, name="k_ps", tag="bank")
            for kt in range(4):
                nc.tensor.matmul(
                    k_ps[:],
                    wk2n[:, kt, :],
                    hidT_par(kt, par),
                    start=(kt == 0), stop=(kt == 3),
                )
            kps.append(k_ps)
        for h in range(2):
            k_sb = consts.tile([128, 256], F32, name=f"ktp_{h}")
            for par in range(2):
                if par == 0 or KTP_ACT:
                    nc.scalar.activation(
                        k_sb[64 * par : 64 * par + 64, :],
                        kps[par][64 * h : 64 * h + 64, :],
                        Act.Identity,
                        bias=bkncol[64 * par : 64 * par + 64, h : h + 1],
                        scale=1.0,
                    )
                else:
                    nc.vector.tensor_scalar(
                        k_sb[64 * par : 64 * par + 64, :],
                        kps[par][64 * h : 64 * h + 64, :],
                        bkncol[64 * par : 64 * par + 64, h : h + 1],
                        None, Alu.add,
                    )
            ktp.append(k_sb)

    emit_q(0)
    emit_k()
    emit_q(1)

    def emit_ksum():
        # Ksum bias columns: ksr rows (pre-scaled by -SC on host) -> columns
        ks_ps = paux.tile([2, 512], F32, name="ks_ps", tag="aux")
        for kt in range(4):
            nc.tensor.matmul(
                ks_ps[:], wks2[:, kt, :], hidT[:, kt, :],
                start=(kt == 0), stop=(kt == 3),
            )
        ksr = consts.tile([2, 512], F32, name="ksr")
        nc.vector.tensor_scalar(
            ksr[:], ks_ps[:], bks2[:, 0:1], None, Alu.add
        )
        kscol_ps = paux.tile([128, 4, 2], F32, name="kscol_ps", tag="aux")
        for bk in range(4):
            nc.tensor.matmul(
                kscol_ps[:, bk, :],
                ksr[:, 128 * bk : 128 * bk + 128],
                eye2[:],
                start=True, stop=True,
            )
        nc.vector.tensor_copy(kscol[:], kscol_ps[:])

    def emit_v():
        for jt in range(4):
            v_ps = pbank.tile([128, 128], F32, name="v_ps", tag="bank")
            for kt in range(4):
                nc.tensor.matmul(
                    v_ps[:], hidT[:, kt, jt * 128 : (jt + 1) * 128],
                    wv2[:, kt, :],
                    start=(kt == 0), stop=(kt == 3),
                )
            vdst = vext[:, jt].rearrange("p (three k) -> p three k", three=3)[:, 0:3:2, :]
            vsrc = v_ps[:].rearrange("p (two k) -> p two k", two=2)
            nc.vector.tensor_copy(vdst, vsrc)

    # ---- distance banks with fp16/fp8 routing ----
    kscol = consts.tile([128, 4, 2], F32, name="kscol")
    et_sb = [consts.tile([128, 4, 512], F16, name=f"et_{h}") for h in range(2)]
    route_state = {"ctr": 0}
    eng_load = {"pool": LOAD0_POOL, "act": LOAD0_ACT, "dve8": LOAD0_DVE}
    pending_exp = []
    av_ps = {}

    def flush_exp():
        while pending_exp:
            ph, pbk, pdt = pending_exp.pop(0)
            nc.scalar.activation(
                et_sb[ph][:, pbk, :], pdt[:], Act.Exp,
                bias=kscol[:, pbk, ph : ph + 1], scale=-SCALE,
            )
            eng_load["act"] += 600
            if AV_LATE:
                continue
            # pipeline the av|cs accumulation for this bank right behind its exp
            if ph not in av_ps:
                av_ps[ph] = paux.tile([128, 512], F32, name=f"av_ps{ph}", tag="aux")
            nc.tensor.matmul(
                av_ps[ph][:], vext[:, pbk, 64 * ph : 64 * ph + 128],
                et_sb[ph][:, pbk, :],
                start=(pbk == 0), stop=(pbk == 3),
            )

    def pick_engine():
        e = min(("pool", "act", "dve8"),
                key=lambda x: eng_load[x] + {"pool": COST_POOL, "act": COST_ACT,
                                             "dve8": COST_DVE8}[x])
        eng_load[e] += {"pool": COST_POOL, "act": COST_ACT, "dve8": COST_DVE8}[e]
        return e

    def emit_bank(h, bk):
        dt_ps = pbank.tile([128, 512], F32, name="dt_ps", tag="bank")
        # route each of the 32 slots (g, tp); counter keeps exact global ratio
        slots = [(g, tp) for tp in range(8) for g in range(4)]
        modes = {}
        for s in slots:
            c = route_state["ctr"]
            modes[s] = ((c * F8_NUM) // F8_DEN) != (((c + 1) * F8_NUM) // F8_DEN)
            route_state["ctr"] = c + 1
        order = list(slots)
        first8 = next((s for s in order if modes[s]), None)
        if first8 is not None:
            order.remove(first8)
            order.insert(0, first8)
        mixed = first8 is not None
        n_mm = 0
        total_mm = sum(1 if modes[s] else 2 for s in slots)
        g_seen = set()
        g_count = {g: 0 for g in range(4)}
        g_total = {g: sum(1 if modes[(g, tp)] else 2 for tp in range(8)) for g in range(4)}
        for si, (g, tp) in enumerate(order):
            if si == 12:
                flush_exp()
            if modes[(g, tp)]:
                apair = work.tile([128, 2, 512], F8, name="ap8", tag="a")
                for i in range(2):
                    jp = 64 * bk + 16 * g + 2 * tp + i
                    e = pick_engine()
                    if e == "act":
                        nc.scalar.activation(
                            apair[:, i, :], qt2[h][:], Act.Relu,
                            bias=ktp[h][:, jp : jp + 1], scale=1.0,
                        )
                    elif e == "pool":
                        nc.gpsimd.tensor_scalar(
                            apair[:, i, :], qt2[h][:], ktp[h][:, jp : jp + 1],
                            0.0, Alu.add, Alu.max,
                        )
                    else:
                        nc.vector.tensor_scalar(
                            apair[:, i, :], qt2[h][:], ktp[h][:, jp : jp + 1],
                            0.0, Alu.add, Alu.max,
                        )
                nc.tensor.matmul(
                    dt_ps[:], sel8[:, 8 * g + tp], apair[:],
                    start=(n_mm == 0), stop=(n_mm == total_mm - 1),
                    perf_mode=DR, skip_group_check=True,
                )
                n_mm += 1
                g_seen.add(g)
            else:
                for i in range(2):
                    t = 2 * tp + i
                    jp = 64 * bk + 16 * g + t
                    a = work.tile([128, 512], F16, name="a", tag="a")
                    nc.vector.tensor_scalar(
                        a[:], qt2[h][:], ktp[h][:, jp : jp + 1], 0.0,
                        Alu.add, Alu.max,
                    )
                    eng_load["dve8"] += COST_DVE16
                    if mixed:
                        st = (n_mm == 0)
                        sp = (n_mm == total_mm - 1)
                    else:
                        st = g_count[g] == 0
                        sp = g_count[g] == g_total[g] - 1
                    nc.tensor.matmul(
                        dt_ps[32 * g : 32 * g + 32, :],
                        sel16[:, t, :], a[:],
                        start=st, stop=sp,
                        tile_position=(0, 32 * g),
                        skip_group_check=mixed,
                    )
                    n_mm += 1
                    g_count[g] += 1
        pending_exp.append((h, bk, dt_ps))

    emit_bank(0, 0)
    emit_ksum()
    emit_bank(0, 1)
    emit_v()
    emit_bank(0, 2)
    emit_bank(0, 3)
    emit_bank(1, 0)

    # ---- softmax normalize (av|cs fused via vext ones block) ----
    normT = consts.tile([128, 512], F16, name="normT")

    def emit_norm(h, half):
        # h=0: rows 0:64 av, 64:128 cs ; h=1: rows 0:64 cs, 64:128 av
        cs_rows = slice(64, 128) if h == 0 else slice(0, 64)
        av_rows = slice(0, 64) if h == 0 else slice(64, 128)
        cols = slice(256 * half, 256 * half + 256)
        recip = recips[h]
        nc.vector.reciprocal(recip[:, cols], av_ps[h][cs_rows, cols])
        nc.vector.tensor_mul(
            normT[64 * h : 64 * h + 64, cols], av_ps[h][av_rows, cols],
            recip[:, cols],
        )

    def emit_av_block(h):
        av_ps[h] = paux.tile([128, 512], F32, name=f"av_ps{h}", tag="aux")
        for jt in range(4):
            nc.tensor.matmul(
                av_ps[h][:], vext[:, jt, 64 * h : 64 * h + 128],
                et_sb[h][:, jt, :],
                start=(jt == 0), stop=(jt == 3),
            )

    recips = [consts.tile([64, 512], F32, name=f"recip{h}") for h in range(2)]
    if AV_LATE:
        emit_av_block(0)
    emit_norm(0, 0)
    emit_bank(1, 1)
    emit_norm(0, 1)
    emit_bank(1, 2)
    emit_bank(1, 3)
    flush_exp()
    if AV_LATE:
        emit_av_block(1)

    def emit_outproj(st):
        f_ps = pbank.tile([128, 512], F32, name="f_ps", tag="bank")
        nc.tensor.matmul(
            f_ps[:], normT[:, st * 128 : (st + 1) * 128], wo2[:],
            start=True, stop=True,
        )
        o_sb = work.tile([128, 512], F32, name="o_sb", tag="o")
        if st % 2 == 0:
            nc.vector.tensor_copy(o_sb[:], f_ps[:])
        else:
            nc.scalar.copy(o_sb[:], f_ps[:])
        nc.sync.dma_start(outp_d[st * 128 : (st + 1) * 128, :], o_sb[:])

    emit_norm(1, 0)
    emit_outproj(0)
    emit_outproj(1)
    emit_norm(1, 1)
    emit_outproj(2)
    emit_outproj(3)


_NC = None


def _get_nc():
    global _NC
    if _NC is None:
        _NC = _build_program()
    return _NC


def _host_constants():
    import ml_dtypes
    sel16 = np.zeros((128, 16, 32), np.float16)
    for t in range(16):
        for p in range(128):
            sel16[p, t, 2 * t + p // 64] = 2.0
    sel8 = np.zeros((128, 32, 2, 128), ml_dtypes.float8_e4m3)
    for g in range(4):
        for tp in range(8):
            for i in range(2):
                for p in range(128):
                    sel8[p, 8 * g + tp, i, 32 * g + 4 * tp + 2 * i + p // 64] = 2.0
    return sel16, sel8


def kernel(hidden_states, Wq, bq, Wk, bk, Wv, bv, Wo, bo):
    hidden_states = np.asarray(hidden_states, np.float32)
    Wq, bq = np.asarray(Wq, np.float32), np.asarray(bq, np.float32)
    Wk, bk = np.asarray(Wk, np.float32), np.asarray(bk, np.float32)
    Wv, bv = np.asarray(Wv, np.float32), np.asarray(bv, np.float32)
    Wo, bo = np.asarray(Wo, np.float32), np.asarray(bo, np.float32)

    sel16, sel8 = _host_constants()
    in_maps = []
    for core in range(N_CORES):
        b = core // 4
        hp = core % 4
        cols = slice(hp * 128, hp * 128 + 128)
        wk_sl = Wk[:, cols]
        bq_sl, bk_sl = bq[cols.start : cols.stop], bk[cols.start : cols.stop]
        bqcol = np.stack(
            [np.tile(bq_sl[lh * 64 : lh * 64 + 64], 2) for lh in range(2)], axis=1
        ).astype(np.float32)
        bkncol = np.stack(
            [np.tile(-bk_sl[lh * 64 : lh * 64 + 64], 2) for lh in range(2)], axis=1
        ).astype(np.float32)
        bks2 = np.array(
            [[-SCALE * bk_sl[0:64].sum()], [-SCALE * bk_sl[64:128].sum()]], np.float32
        )
        in_maps.append(
            {
                "hidt": np.ascontiguousarray(hidden_states[b].T).astype(np.float16),
                "wq2": np.concatenate(
                    [
                        np.concatenate([Wq[:, cols][:, l * 64 : l * 64 + 64]] * 2, axis=1)
                        for l in range(2)
                    ],
                    axis=1,
                ).astype(np.float16),
                "wk2n": np.ascontiguousarray(-wk_sl).astype(np.float16),
                "wv2": np.ascontiguousarray(Wv[:, cols]).astype(np.float16),
                "wo2": np.ascontiguousarray(Wo[cols, :]).astype(np.float16),
                "wks2": (-SCALE * wk_sl.reshape(HID, 2, 64).sum(-1)).astype(np.float16),
                "bqcol": bqcol,
                "bkncol": bkncol,
                "bks2": bks2,
                "sel16": sel16,
                "sel8": sel8,
                "eye2": np.eye(2, dtype=np.float32),
            }
        )

    nc = _get_nc()
    res = run_bass_kernel_spmd(nc, in_maps, core_ids=list(range(N_CORES)))
    parts = [r["outp"] for r in res.results]
    bo_eff = bv @ Wo + bo
    out = np.stack(
        [
            parts[0] + parts[1] + parts[2] + parts[3],
            parts[4] + parts[5] + parts[6] + parts[7],
        ],
        axis=0,
    )
    return (out + bo_eff[None, None, :]).astype(np.float32)


if __name__ == "__main__":
    rng = np.random.default_rng(0)
    w = 0.02
    inputs = {
        "hidden_states": rng.standard_normal((B, S, HID)).astype(np.float32),
        "Wq": (rng.standard_normal((HID, HID)) * w).astype(np.float32),
        "bq": np.zeros(HID, np.float32),
        "Wk": (rng.standard_normal((HID, HID)) * w).astype(np.float32),
        "bk": np.zeros(HID, np.float32),
        "Wv": (rng.standard_normal((HID, HID)) * w).astype(np.float32),
        "bv": np.zeros(HID, np.float32),
        "Wo": (rng.standard_normal((HID, HID)) * w).astype(np.float32),
        "bo": np.zeros(HID, np.float32),
    }
    out = kernel(**inputs)
    print("out shape:", out.shape, "finite:", np.isfinite(out).all())
